# revision 14
# baseline (speedup 1.0000x reference)
"""Conv2d 3x3 VALID stride-1 kernel for Trainium2 (Bass/Tile), 8-core SPMD.

x: [32, 128, 112, 112] f32, weight: [256, 128, 3, 3] f32
out: [32, 256, 110, 110] f32

Strategy: implicit GEMM. Cin=128 sits on the SBUF partition dim and is the
matmul contraction axis. For each of the 9 filter taps (kh, kw), a matmul
with lhsT = weight[ci, co_tile] and rhs = x[ci, shifted-window pixels]
accumulates into PSUM (start on tap 0). Output row-chunks of 4 rows
(free dim 440 <= 512 = one PSUM bank) stream through the PE at 1 fp32r
cycle/row. Data-parallel over batch: 4 images per core.
"""

import numpy as np

import concourse.bass as bass
import concourse.mybir as mybir
import concourse.tile as tile
from concourse import bacc
from concourse.bass_utils import run_bass_kernel_spmd

B, CIN, H, W = 32, 128, 112, 112
COUT, KH, KW = 256, 3, 3
OH, OW = H - KH + 1, W - KW + 1  # 110, 110
NCORES = 8
BPC = B // NCORES  # batches per core

F32 = mybir.dt.float32
F32R = mybir.dt.float32r
BF16 = mybir.dt.bfloat16

# Compute dtype for the TensorEngine inputs. float32r (TF32-like, 10-bit
# mantissa) gives rel err ~1.5e-4 at 200 ns/MM; bfloat16 gives ~2e-3 at
# 186 ns/MM (LDWEIGHTS fully hidden).
import os as _os
FP16 = mybir.dt.float16
_DT_MAP = {"f32r": F32R, "bf16": BF16, "fp16": FP16}
COMPUTE_DT = _DT_MAP[_os.environ.get("CONV_DT", "fp16")]

# Row-chunking of the 110 output rows: free dim = rows*110, must be <= 512
# and >= 256 (fp32r full-rate threshold). 26*4 + 2*3 = 110.
ROW_CHUNKS = [4] * 26 + [3] * 2

_CACHE = {}


def _build_nc():
    nc = bacc.Bacc("TRN2", target_bir_lowering=False, debug=False)

    x_d = nc.dram_tensor("x", [BPC, CIN, H, W], F32, kind="ExternalInput")
    w_d = nc.dram_tensor("w", [CIN, KH * KW, COUT], F32, kind="ExternalInput")
    o_d = nc.dram_tensor("o", [BPC, COUT, OH, OW], F32, kind="ExternalOutput")

    from concourse.bass import _add_dep_helper

    xbufs = 2 if COMPUTE_DT == F32R else 3
    # Prefetch chunking of images b >= 1 (14-row pieces), paced against the
    # previous batch's compute so the SWDGE input stream never bursts hard
    # enough to starve the HWDGE output stores of SDMA bandwidth.
    PF_BOUNDS = [0, 14, 28, 42, 56, 70, 84, 98, 112]
    N_GROUPS = 2 * len(ROW_CHUNKS)  # (row-chunk, ct) groups per batch

    with tile.TileContext(nc) as tc:
        with (
            tc.tile_pool(name="wpool", bufs=1) as wpool,
            tc.tile_pool(name="xpool", bufs=xbufs) as xpool,
            tc.tile_pool(name="opool", bufs=16) as opool,
            tc.tile_pool(name="psum", bufs=8, space="PSUM") as psum,
        ):
            # PE pre-warm: dependency-free dummy matmuls on a never-written
            # scratch tile keep the PE busy from engine boot until the first
            # real matmul's data arrives, so the HAM clock gate is already
            # at 2.4 GHz (warm) when real work starts and the ~3 us
            # half-clock ramp is paid on garbage instead.
            scratch = wpool.tile([128, 512], COMPUTE_DT, name="warm_scratch")
            nc.vector.memset(scratch[:], 0)
            ps_warm = psum.tile([128, 512], F32, name="warm_psum", tag="ps")
            for _ in range(48):
                nc.tensor.matmul(
                    ps_warm[:], scratch[:, 0:128], scratch[:],
                    start=True, stop=True, skip_group_check=True,
                )

            wr = wpool.tile([CIN, KH * KW, COUT], COMPUTE_DT)
            # ct=0's weight columns first: the first matmuls need only them.
            nc.gpsimd.dma_start(wr[:, :, 0:128], w_d[:, :, 0:128])

            # Image 0: load immediately (it gates the first matmuls). Small
            # leading chunk = exactly the rows the first matmul group reads.
            xtiles = [xpool.tile([CIN, H, W], COMPUTE_DT, tag="x", name="x0")]
            for r0, r1 in zip(b0 := [0, 6, 16, 28, 42, 56, 70, 84, 98, 112], b0[1:]):
                nc.gpsimd.dma_start(
                    xtiles[0][:, r0:r1, :], x_d[0, :, r0:r1, :]
                )
                if r1 == 6:
                    nc.gpsimd.dma_start(wr[:, :, 128:256], w_d[:, :, 128:256])

            for b in range(BPC):
                xr = xtiles[b]
                if b + 1 < BPC:
                    xtiles.append(
                        xpool.tile(
                            [CIN, H, W], COMPUTE_DT, tag="x", name=f"x{b+1}"
                        )
                    )
                # Milestone group index at which to release prefetch chunk j
                # of image b+1: spread the 8 chunks across this batch.
                pf_at = {
                    (N_GROUPS * j) // len(PF_BOUNDS[1:]): j
                    for j in range(len(PF_BOUNDS) - 1)
                }

                # Interleave the two cout-tiles per row-chunk: halves the
                # x-row consumption rate so compute never overruns the
                # image DMA at kernel start.
                oh = 0
                gidx = 0
                for R in ROW_CHUNKS:
                    for ct in range(2):
                        co0 = ct * 128
                        ps = psum.tile([128, R, OW], F32, tag="ps")
                        for idx in range(KH * KW):
                            kh, kw = divmod(idx, KW)
                            nc.tensor.matmul(
                                ps[:],
                                wr[:, idx, co0 : co0 + 128],
                                xr[:, oh + kh : oh + kh + R, kw : kw + OW],
                                start=(idx == 0),
                                stop=(idx == KH * KW - 1),
                            )
                        ot = opool.tile([128, R, OW], F32, tag="ot")
                        cp = nc.vector.tensor_copy(ot[:], ps[:])
                        nc.sync.dma_start(
                            o_d[b, co0 : co0 + 128, oh : oh + R, :], ot[:]
                        )
                        if b + 1 < BPC and gidx in pf_at:
                            j = pf_at[gidx]
                            r0, r1 = PF_BOUNDS[j], PF_BOUNDS[j + 1]
                            dma = nc.gpsimd.dma_start(
                                xtiles[b + 1][:, r0:r1, :],
                                x_d[b + 1, :, r0:r1, :],
                            )
                            _add_dep_helper(
                                dma.ins,
                                cp.ins,
                                sync=True,
                                reason="pace input prefetch vs compute",
                            )
                        gidx += 1
                    oh += R

    nc.compile()
    return nc


def _get_nc():
    if "nc" not in _CACHE:
        _CACHE["nc"] = _build_nc()
    return _CACHE["nc"]


LAST_RESULT = None


def kernel(x, weight, trace=False):
    global LAST_RESULT
    x = np.ascontiguousarray(np.asarray(x, dtype=np.float32))
    weight = np.asarray(weight, dtype=np.float32)
    # [Cout, Cin, kh, kw] -> [Cin, kh*kw, Cout], contiguous
    w_packed = np.ascontiguousarray(
        weight.transpose(1, 2, 3, 0).reshape(CIN, KH * KW, COUT)
    )

    nc = _get_nc()
    in_maps = [
        {"x": x[i * BPC : (i + 1) * BPC], "w": w_packed} for i in range(NCORES)
    ]
    res = run_bass_kernel_spmd(
        nc, in_maps, core_ids=list(range(NCORES)), trace=trace
    )
    LAST_RESULT = res
    out = np.concatenate([r["o"] for r in res.results], axis=0)
    return out


# revision 15
# speedup vs baseline: 1.0173x; 1.0173x over previous
"""Conv2d 3x3 VALID stride-1 kernel for Trainium2 (Bass/Tile), 8-core SPMD.

x: [32, 128, 112, 112] f32, weight: [256, 128, 3, 3] f32
out: [32, 256, 110, 110] f32

Strategy: implicit GEMM. Cin=128 sits on the SBUF partition dim and is the
matmul contraction axis. For each of the 9 filter taps (kh, kw), a matmul
with lhsT = weight[ci, co_tile] and rhs = x[ci, shifted-window pixels]
accumulates into PSUM (start on tap 0). Output row-chunks of 4 rows
(free dim 440 <= 512 = one PSUM bank) stream through the PE at 1 fp32r
cycle/row. Data-parallel over batch: 4 images per core.
"""

import numpy as np

import concourse.bass as bass
import concourse.mybir as mybir
import concourse.tile as tile
from concourse import bacc
from concourse.bass_utils import run_bass_kernel_spmd

B, CIN, H, W = 32, 128, 112, 112
COUT, KH, KW = 256, 3, 3
OH, OW = H - KH + 1, W - KW + 1  # 110, 110
NCORES = 8
BPC = B // NCORES  # batches per core

F32 = mybir.dt.float32
F32R = mybir.dt.float32r
BF16 = mybir.dt.bfloat16

# Compute dtype for the TensorEngine inputs. float32r (TF32-like, 10-bit
# mantissa) gives rel err ~1.5e-4 at 200 ns/MM; bfloat16 gives ~2e-3 at
# 186 ns/MM (LDWEIGHTS fully hidden).
import os as _os
FP16 = mybir.dt.float16
_DT_MAP = {"f32r": F32R, "bf16": BF16, "fp16": FP16}
COMPUTE_DT = _DT_MAP[_os.environ.get("CONV_DT", "fp16")]

# Row-chunking of the 110 output rows: free dim = rows*110, must be <= 512
# and >= 256 (fp32r full-rate threshold). 26*4 + 2*3 = 110.
ROW_CHUNKS = [4] * 26 + [3] * 2

_CACHE = {}


def _build_nc():
    nc = bacc.Bacc("TRN2", target_bir_lowering=False, debug=False)

    x_d = nc.dram_tensor("x", [BPC, CIN, H, W], F32, kind="ExternalInput")
    w_d = nc.dram_tensor("w", [CIN, KH * KW, COUT], F32, kind="ExternalInput")
    o_d = nc.dram_tensor("o", [BPC, COUT, OH, OW], F32, kind="ExternalOutput")

    from concourse.bass import _add_dep_helper

    xbufs = 2 if COMPUTE_DT == F32R else 3
    # Prefetch chunking of images b >= 1 (14-row pieces), paced against the
    # previous batch's compute so the SWDGE input stream never bursts hard
    # enough to starve the HWDGE output stores of SDMA bandwidth.
    PF_BOUNDS = [0, 14, 28, 42, 56, 70, 84, 98, 112]
    N_GROUPS = 2 * len(ROW_CHUNKS)  # (row-chunk, ct) groups per batch

    with tile.TileContext(nc) as tc:
        with (
            tc.tile_pool(name="wpool", bufs=1) as wpool,
            tc.tile_pool(name="xpool", bufs=xbufs) as xpool,
            tc.tile_pool(name="opool", bufs=16) as opool,
            tc.tile_pool(name="psum", bufs=8, space="PSUM") as psum,
        ):
            # PE pre-warm: dependency-free dummy matmuls on a never-written
            # scratch tile keep the PE busy from engine boot until the first
            # real matmul's data arrives, so the HAM clock gate is already
            # at 2.4 GHz (warm) when real work starts and the ~3 us
            # half-clock ramp is paid on garbage instead.
            scratch = wpool.tile([128, 512], COMPUTE_DT, name="warm_scratch")
            nc.vector.memset(scratch[:], 0)
            ps_warm = psum.tile([128, 512], F32, name="warm_psum", tag="ps")
            for _ in range(16):
                nc.tensor.matmul(
                    ps_warm[:], scratch[:, 0:128], scratch[:],
                    start=True, stop=True, skip_group_check=True,
                )

            wr = wpool.tile([CIN, KH * KW, COUT], COMPUTE_DT)
            # ct=0's weight columns first: the first matmuls need only them.
            nc.gpsimd.dma_start(wr[:, :, 0:128], w_d[:, :, 0:128])

            # Image 0: load immediately (it gates the first matmuls). Small
            # leading chunk = exactly the rows the first matmul group reads.
            xtiles = [xpool.tile([CIN, H, W], COMPUTE_DT, tag="x", name="x0")]
            for r0, r1 in zip(b0 := [0, 6, 16, 28, 42, 56, 70, 84, 98, 112], b0[1:]):
                nc.gpsimd.dma_start(
                    xtiles[0][:, r0:r1, :], x_d[0, :, r0:r1, :]
                )
                if r1 == 6:
                    nc.gpsimd.dma_start(wr[:, :, 128:256], w_d[:, :, 128:256])

            for b in range(BPC):
                xr = xtiles[b]
                if b + 1 < BPC:
                    xtiles.append(
                        xpool.tile(
                            [CIN, H, W], COMPUTE_DT, tag="x", name=f"x{b+1}"
                        )
                    )
                # Milestone group index at which to release prefetch chunk j
                # of image b+1: spread the 8 chunks across this batch.
                pf_at = {
                    (N_GROUPS * j) // len(PF_BOUNDS[1:]): j
                    for j in range(len(PF_BOUNDS) - 1)
                }

                # Interleave the two cout-tiles per row-chunk: halves the
                # x-row consumption rate so compute never overruns the
                # image DMA at kernel start.
                oh = 0
                gidx = 0
                for R in ROW_CHUNKS:
                    for ct in range(2):
                        co0 = ct * 128
                        ps = psum.tile([128, R, OW], F32, tag="ps")
                        for idx in range(KH * KW):
                            kh, kw = divmod(idx, KW)
                            nc.tensor.matmul(
                                ps[:],
                                wr[:, idx, co0 : co0 + 128],
                                xr[:, oh + kh : oh + kh + R, kw : kw + OW],
                                start=(idx == 0),
                                stop=(idx == KH * KW - 1),
                            )
                        ot = opool.tile([128, R, OW], F32, tag="ot")
                        cp = nc.vector.tensor_copy(ot[:], ps[:])
                        nc.sync.dma_start(
                            o_d[b, co0 : co0 + 128, oh : oh + R, :], ot[:]
                        )
                        if b + 1 < BPC and gidx in pf_at:
                            j = pf_at[gidx]
                            r0, r1 = PF_BOUNDS[j], PF_BOUNDS[j + 1]
                            dma = nc.gpsimd.dma_start(
                                xtiles[b + 1][:, r0:r1, :],
                                x_d[b + 1, :, r0:r1, :],
                            )
                            _add_dep_helper(
                                dma.ins,
                                cp.ins,
                                sync=True,
                                reason="pace input prefetch vs compute",
                            )
                        gidx += 1
                    oh += R

    nc.compile()
    return nc


def _get_nc():
    if "nc" not in _CACHE:
        _CACHE["nc"] = _build_nc()
    return _CACHE["nc"]


LAST_RESULT = None


def kernel(x, weight, trace=False):
    global LAST_RESULT
    x = np.ascontiguousarray(np.asarray(x, dtype=np.float32))
    weight = np.asarray(weight, dtype=np.float32)
    # [Cout, Cin, kh, kw] -> [Cin, kh*kw, Cout], contiguous
    w_packed = np.ascontiguousarray(
        weight.transpose(1, 2, 3, 0).reshape(CIN, KH * KW, COUT)
    )

    nc = _get_nc()
    in_maps = [
        {"x": x[i * BPC : (i + 1) * BPC], "w": w_packed} for i in range(NCORES)
    ]
    res = run_bass_kernel_spmd(
        nc, in_maps, core_ids=list(range(NCORES)), trace=trace
    )
    LAST_RESULT = res
    out = np.concatenate([r["o"] for r in res.results], axis=0)
    return out
